# revision 10
# baseline (speedup 1.0000x reference)
"""Trainium2 Bass kernel for AttentionFusion3.

Computes, for A, B in [D, D] (D=8192), w in [D], b in [1]:
    diag = sum(A * B, axis=-1)            # [D] row-wise dot products
    aw   = tanh(dot(diag, w) + b[0])      # scalar gate
    out  = A * aw + B * (1 - aw)

Sharding: rows split across 8 NeuronCores (1024 rows each). Each core:
  pass 1: stream its A/B shard, fused multiply+row-reduce (DVE
          tensor_tensor_reduce), fold in w, partition-reduce via a
          ones-matmul on PE (which also broadcasts the partial to all
          128 partitions and adds b/8), AllReduce the [128,1] partial
          vector across the 8 cores (sum of per-core partials + 8*(b/8)
          = dot + b, replicated on every partition),
  gate:   tanh on ScalarE, 1-gate,
  pass 2: re-stream A/B, out = gate*A + (1-gate)*B via two ScalarE
          scales + one DVE add, store.
"""

import numpy as np

try:
    import concourse.bass as bass
except ImportError:  # fallback for environments without concourse on sys.path
    import sys

    for _p in ("/opt/trn_rl_repo", "/root/.axon_site/_ro/trn_rl_repo"):
        sys.path.insert(0, _p)
    import concourse.bass as bass  # noqa: F401

import concourse.bacc as bacc
import concourse.mybir as mybir
from concourse import tile
from concourse.bass_utils import run_bass_kernel_spmd

N_CORES = 8
P = 128
D = 8192
ROWS = D // N_CORES  # 1024 rows per core


def build_nc(rows=ROWS, d=D, cw=4096, n_cores=N_CORES):
    """Build + compile the per-core Bass program.

    rows: rows per core; d: row length; cw: column chunk width.
    """
    rb_n = rows // P  # row blocks per core
    cc_n = d // cw  # column chunks per row block

    fp32 = mybir.dt.float32
    Alu = mybir.AluOpType
    Act = mybir.ActivationFunctionType

    nc = bacc.Bacc(
        "TRN2", target_bir_lowering=False, debug=False, num_devices=n_cores
    )
    A = nc.dram_tensor("A", [rows, d], fp32, kind="ExternalInput")
    Bm = nc.dram_tensor("B", [rows, d], fp32, kind="ExternalInput")
    W = nc.dram_tensor("w", [P, rb_n], fp32, kind="ExternalInput")
    B8 = nc.dram_tensor("b8", [1, 1], fp32, kind="ExternalInput")
    OUT = nc.dram_tensor("out", [rows, d], fp32, kind="ExternalOutput")

    with tile.TileContext(nc) as tc:
        with (
            tc.tile_pool(name="a_pool", bufs=5) as a_pool,
            tc.tile_pool(name="b_pool", bufs=5) as b_pool,
            tc.tile_pool(name="small", bufs=1) as small,
            tc.tile_pool(name="dram", bufs=1, space="DRAM") as dram,
            tc.tile_pool(name="psum", bufs=1, space="PSUM") as psum_pool,
        ):
            w_t = small.tile([P, rb_n], fp32)
            b8_t = small.tile([1, 1], fp32)
            diagcols = small.tile([P, rb_n * cc_n], fp32)
            comb = small.tile([P, rb_n], fp32)
            sred = small.tile([P, 1], fp32)
            ones_t = small.tile([P, P], fp32)
            gathered = small.tile([P, n_cores], fp32)
            pre_gate = small.tile([P, 1], fp32)
            gate = small.tile([P, 1], fp32)
            omg = small.tile([P, 1], fp32)
            cc_sb = small.tile([P, 1], fp32)
            cc_in = dram.tile([P, 1], fp32)
            cc_out = dram.tile([P * n_cores, 1], fp32, addr_space="Shared")
            ps = psum_pool.tile([P, 1], fp32)

            nc.sync.dma_start(w_t[:], W[:])
            nc.sync.dma_start(b8_t[:], B8[:])
            nc.vector.memset(ones_t[:], 1.0)

            # ---- pass 1: diagcols[:, cc*rb_n + rb] = rowsum(A_chunk * B_chunk)
            for rb in range(rb_n):
                for cc in range(cc_n):
                    a_t = a_pool.tile([P, cw], fp32)
                    b_t = b_pool.tile([P, cw], fp32)
                    rs = slice(rb * P, (rb + 1) * P)
                    cs = slice(cc * cw, (cc + 1) * cw)
                    nc.sync.dma_start(a_t[:], A[rs, cs])
                    nc.sync.dma_start(b_t[:], Bm[rs, cs])
                    k = cc * rb_n + rb
                    # fused A*B + row-sum in one DVE op (product written
                    # in-place over the A tile and discarded)
                    nc.vector.scalar_tensor_tensor(
                        out=a_t[:],
                        in0=a_t[:],
                        scalar=1.0,
                        in1=b_t[:],
                        op0=Alu.mult,
                        op1=Alu.mult,
                        accum_out=diagcols[:, k : k + 1],
                    )

            # combine column chunks: comb[:, rb] = sum_cc diagcols[:, cc*rb_n+rb]
            if cc_n == 1:
                nc.vector.tensor_copy(comb[:], diagcols[:])
            else:
                nc.vector.tensor_add(
                    comb[:], diagcols[:, 0:rb_n], diagcols[:, rb_n : 2 * rb_n]
                )
                for cc in range(2, cc_n):
                    nc.vector.tensor_add(
                        comb[:], comb[:], diagcols[:, cc * rb_n : (cc + 1) * rb_n]
                    )
            # fold in w and reduce to per-partition partial
            nc.vector.tensor_mul(comb[:], comb[:], w_t[:])
            nc.vector.reduce_sum(sred[:], comb[:], axis=mybir.AxisListType.X)

            # partition-sum + broadcast via ones-matmul; accumulate b/8
            nc.tensor.matmul(ps[:], ones_t[:], sred[:], start=True, stop=False)
            nc.tensor.matmul(ps[:], ones_t[0:1, :], b8_t[:], start=False, stop=True)
            nc.scalar.copy(cc_sb[:], ps[:])

            # AllGather the [128,1] partials (all lanes identical per core):
            # out is [n_cores*128, 1] with rank r's block at rows r*128..;
            # AG has a much lower latency floor than AllReduce, and the
            # 8-way sum is a trivial local reduce.
            # cc_in is issued from Scalar's DGE: Sync's queues are full of
            # bulk loads, which would delay this tiny transfer ~30us.
            nc.scalar.dma_start(cc_in[:], cc_sb[:])
            nc.gpsimd.collective_compute(
                "AllGather",
                Alu.bypass,
                replica_groups=[list(range(n_cores))],
                ins=[cc_in.opt()],
                outs=[cc_out.opt()],
            )
            # gathered[p, r] = cc_out[r*128 + p]. Issued from the Scalar
            # sequencer: it waits on the AllGather, and on Sync it would
            # head-of-line-block every pass-2 load queued behind it; Scalar's
            # next op (tanh) depends on this load anyway.
            nc.scalar.dma_start(
                gathered[:], cc_out[:, 0].rearrange("(r p) -> p r", p=P)
            )
            nc.vector.reduce_sum(
                pre_gate[:], gathered[:], axis=mybir.AxisListType.X
            )

            nc.scalar.activation(gate[:], pre_gate[:], Act.Tanh)
            # omg = 1 - gate  (Copy computes scale*in + bias)
            nc.scalar.activation(omg[:], gate[:], Act.Copy, bias=1.0, scale=-1.0)

            # ---- pass 2: out = gate*A + (1-gate)*B
            for rb in range(rb_n):
                for cc in range(cc_n):
                    a_t = a_pool.tile([P, cw], fp32)
                    b_t = b_pool.tile([P, cw], fp32)
                    rs = slice(rb * P, (rb + 1) * P)
                    cs = slice(cc * cw, (cc + 1) * cw)
                    nc.sync.dma_start(a_t[:], A[rs, cs])
                    nc.sync.dma_start(b_t[:], Bm[rs, cs])
                    nc.scalar.mul(a_t[:], a_t[:], gate[:])
                    nc.scalar.mul(b_t[:], b_t[:], omg[:])
                    nc.vector.tensor_add(a_t[:], a_t[:], b_t[:])
                    nc.sync.dma_start(OUT[rs, cs], a_t[:])

    nc.compile()
    return nc


_NC_CACHE = {}


def _get_nc(rows, d, cw, n_cores):
    key = (rows, d, cw, n_cores)
    if key not in _NC_CACHE:
        _NC_CACHE[key] = build_nc(rows, d, cw, n_cores)
    return _NC_CACHE[key]


def make_in_maps(A, B, w, b, n_cores=N_CORES):
    """Shard full inputs row-wise into per-core input maps."""
    rows = A.shape[0] // n_cores
    rb_n = rows // P
    b8 = (b.astype(np.float32) / n_cores).reshape(1, 1)
    in_maps = []
    for c in range(n_cores):
        rs = slice(c * rows, (c + 1) * rows)
        w_core = np.ascontiguousarray(
            w[rs].astype(np.float32).reshape(rb_n, P).T
        )  # [P, rb_n]
        in_maps.append(
            {
                "A": np.ascontiguousarray(A[rs]),
                "B": np.ascontiguousarray(B[rs]),
                "w": w_core,
                "b8": b8,
            }
        )
    return in_maps


def kernel(A, B, w, b, cw=4096, trace=False):
    A = np.asarray(A, dtype=np.float32)
    B = np.asarray(B, dtype=np.float32)
    w = np.asarray(w, dtype=np.float32)
    b = np.asarray(b, dtype=np.float32)
    d = A.shape[1]
    rows = A.shape[0] // N_CORES
    nc = _get_nc(rows, d, cw, N_CORES)
    in_maps = make_in_maps(A, B, w, b, N_CORES)
    res = run_bass_kernel_spmd(
        nc, in_maps, core_ids=list(range(N_CORES)), trace=trace
    )
    out = np.concatenate([r["out"] for r in res.results], axis=0)
    if trace:
        kernel.last_results = res
    return out


# revision 12
# speedup vs baseline: 1.0163x; 1.0163x over previous
"""Trainium2 Bass kernel for AttentionFusion3.

Computes, for A, B in [D, D] (D=8192), w in [D], b in [1]:
    diag = sum(A * B, axis=-1)            # [D] row-wise dot products
    aw   = tanh(dot(diag, w) + b[0])      # scalar gate
    out  = A * aw + B * (1 - aw)

Sharding: rows split across 8 NeuronCores (1024 rows each). Each core:
  pass 1: stream its A/B shard, fused multiply+row-reduce (DVE
          tensor_tensor_reduce), fold in w, partition-reduce via a
          ones-matmul on PE (which also broadcasts the partial to all
          128 partitions and adds b/8), AllReduce the [128,1] partial
          vector across the 8 cores (sum of per-core partials + 8*(b/8)
          = dot + b, replicated on every partition),
  gate:   tanh on ScalarE, 1-gate,
  pass 2: re-stream A/B, out = gate*A + (1-gate)*B via two ScalarE
          scales + one DVE add, store.
"""

import numpy as np

try:
    import concourse.bass as bass
except ImportError:  # fallback for environments without concourse on sys.path
    import sys

    for _p in ("/opt/trn_rl_repo", "/root/.axon_site/_ro/trn_rl_repo"):
        sys.path.insert(0, _p)
    import concourse.bass as bass  # noqa: F401

import concourse.bacc as bacc
import concourse.mybir as mybir
from concourse import tile
from concourse.bass_utils import run_bass_kernel_spmd

N_CORES = 8
P = 128
D = 8192
ROWS = D // N_CORES  # 1024 rows per core


def build_nc(rows=ROWS, d=D, cw=4096, n_cores=N_CORES):
    """Build + compile the per-core Bass program.

    rows: rows per core; d: row length; cw: column chunk width.
    """
    rb_n = rows // P  # row blocks per core
    cc_n = d // cw  # column chunks per row block

    fp32 = mybir.dt.float32
    Alu = mybir.AluOpType
    Act = mybir.ActivationFunctionType

    nc = bacc.Bacc(
        "TRN2", target_bir_lowering=False, debug=False, num_devices=n_cores
    )
    A = nc.dram_tensor("A", [rows, d], fp32, kind="ExternalInput")
    Bm = nc.dram_tensor("B", [rows, d], fp32, kind="ExternalInput")
    W = nc.dram_tensor("w", [P, rb_n], fp32, kind="ExternalInput")
    B8 = nc.dram_tensor("b8", [1, 1], fp32, kind="ExternalInput")
    OUT = nc.dram_tensor("out", [rows, d], fp32, kind="ExternalOutput")

    with tile.TileContext(nc) as tc:
        with (
            tc.tile_pool(name="a_pool", bufs=5) as a_pool,
            tc.tile_pool(name="b_pool", bufs=5) as b_pool,
            tc.tile_pool(name="small", bufs=1) as small,
            tc.tile_pool(name="dram", bufs=1, space="DRAM") as dram,
            tc.tile_pool(name="psum", bufs=1, space="PSUM") as psum_pool,
        ):
            w_t = small.tile([P, rb_n], fp32)
            b8_t = small.tile([1, 1], fp32)
            diagcols = small.tile([P, rb_n * cc_n], fp32)
            comb = small.tile([P, rb_n], fp32)
            sred = small.tile([P, 1], fp32)
            ones_t = small.tile([P, P], fp32)
            gathered = small.tile([P, n_cores], fp32)
            pre_gate = small.tile([P, 1], fp32)
            gate = small.tile([P, 1], fp32)
            omg = small.tile([P, 1], fp32)
            cc_sb = small.tile([P, 1], fp32)
            cc_in = dram.tile([P, 1], fp32)
            cc_out = dram.tile([P * n_cores, 1], fp32, addr_space="Shared")
            ps = psum_pool.tile([P, 1], fp32)

            nc.sync.dma_start(w_t[:], W[:])
            nc.sync.dma_start(b8_t[:], B8[:])
            nc.vector.memset(ones_t[:], 1.0)

            # ---- pass 1: diagcols[:, cc*rb_n + rb] = rowsum(A_chunk * B_chunk)
            for rb in range(rb_n):
                for cc in range(cc_n):
                    a_t = a_pool.tile([P, cw], fp32)
                    b_t = b_pool.tile([P, cw], fp32)
                    rs = slice(rb * P, (rb + 1) * P)
                    cs = slice(cc * cw, (cc + 1) * cw)
                    nc.sync.dma_start(a_t[:], A[rs, cs])
                    nc.sync.dma_start(b_t[:], Bm[rs, cs])
                    k = cc * rb_n + rb
                    # fused A*B + row-sum in one DVE op (product written
                    # in-place over the A tile and discarded)
                    nc.vector.scalar_tensor_tensor(
                        out=a_t[:],
                        in0=a_t[:],
                        scalar=1.0,
                        in1=b_t[:],
                        op0=Alu.mult,
                        op1=Alu.mult,
                        accum_out=diagcols[:, k : k + 1],
                    )

            # combine column chunks: comb[:, rb] = sum_cc diagcols[:, cc*rb_n+rb]
            if cc_n == 1:
                nc.vector.tensor_copy(comb[:], diagcols[:])
            else:
                nc.vector.tensor_add(
                    comb[:], diagcols[:, 0:rb_n], diagcols[:, rb_n : 2 * rb_n]
                )
                for cc in range(2, cc_n):
                    nc.vector.tensor_add(
                        comb[:], comb[:], diagcols[:, cc * rb_n : (cc + 1) * rb_n]
                    )
            # fold in w and reduce to per-partition partial
            nc.vector.tensor_mul(comb[:], comb[:], w_t[:])
            nc.vector.reduce_sum(sred[:], comb[:], axis=mybir.AxisListType.X)

            # partition-sum + broadcast via ones-matmul; accumulate b/8
            nc.tensor.matmul(ps[:], ones_t[:], sred[:], start=True, stop=False)
            nc.tensor.matmul(ps[:], ones_t[0:1, :], b8_t[:], start=False, stop=True)
            nc.scalar.copy(cc_sb[:], ps[:])

            # AllGather the [128,1] partials (all lanes identical per core):
            # out is [n_cores*128, 1] with rank r's block at rows r*128..;
            # AG has a much lower latency floor than AllReduce, and the
            # 8-way sum is a trivial local reduce.
            # cc_in is issued from Scalar's DGE: Sync's queues are full of
            # bulk loads, which would delay this tiny transfer ~30us.
            ccin_inst = nc.scalar.dma_start(cc_in[:], cc_sb[:])
            nc.gpsimd.collective_compute(
                "AllGather",
                Alu.bypass,
                replica_groups=[list(range(n_cores))],
                ins=[cc_in.opt()],
                outs=[cc_out.opt()],
            )
            # gathered[p, r] = cc_out[r*128 + p]. Issued from the Scalar
            # sequencer: it waits on the AllGather, and on Sync it would
            # head-of-line-block every pass-2 load queued behind it; Scalar's
            # next op (tanh) depends on this load anyway.
            nc.scalar.dma_start(
                gathered[:], cc_out[:, 0].rearrange("(r p) -> p r", p=P)
            )
            nc.vector.reduce_sum(
                pre_gate[:], gathered[:], axis=mybir.AxisListType.X
            )

            nc.scalar.activation(gate[:], pre_gate[:], Act.Tanh)
            # omg = 1 - gate  (Copy computes scale*in + bias)
            nc.scalar.activation(omg[:], gate[:], Act.Copy, bias=1.0, scale=-1.0)

            # ---- pass 2: out = gate*A + (1-gate)*B
            first_pass2_load = True
            for rb in range(rb_n):
                for cc in range(cc_n):
                    a_t = a_pool.tile([P, cw], fp32)
                    b_t = b_pool.tile([P, cw], fp32)
                    rs = slice(rb * P, (rb + 1) * P)
                    cs = slice(cc * cw, (cc + 1) * cw)
                    la = nc.sync.dma_start(a_t[:], A[rs, cs])
                    lb = nc.sync.dma_start(b_t[:], Bm[rs, cs])
                    if first_pass2_load:
                        # Hold pass-2 prefetch until the tiny collective-input
                        # DMA has landed: otherwise ~20MB of queued prefetch
                        # descriptors delay cc_in (and thus every rank's
                        # arrival at the AllGather) by ~40us. The Sync stream
                        # is in-order, so gating the first pair gates all.
                        import bass_rust as _br

                        _br.add_dep_helper(
                            la.ins, ccin_inst.ins, sync=True,
                            reason="drain DGE backlog before collective input",
                        )
                        _br.add_dep_helper(
                            lb.ins, ccin_inst.ins, sync=True,
                            reason="drain DGE backlog before collective input",
                        )
                        first_pass2_load = False
                    nc.scalar.mul(a_t[:], a_t[:], gate[:])
                    nc.scalar.mul(b_t[:], b_t[:], omg[:])
                    nc.vector.tensor_add(a_t[:], a_t[:], b_t[:])
                    nc.sync.dma_start(OUT[rs, cs], a_t[:])

    nc.compile()
    return nc


_NC_CACHE = {}


def _get_nc(rows, d, cw, n_cores):
    key = (rows, d, cw, n_cores)
    if key not in _NC_CACHE:
        _NC_CACHE[key] = build_nc(rows, d, cw, n_cores)
    return _NC_CACHE[key]


def make_in_maps(A, B, w, b, n_cores=N_CORES):
    """Shard full inputs row-wise into per-core input maps."""
    rows = A.shape[0] // n_cores
    rb_n = rows // P
    b8 = (b.astype(np.float32) / n_cores).reshape(1, 1)
    in_maps = []
    for c in range(n_cores):
        rs = slice(c * rows, (c + 1) * rows)
        w_core = np.ascontiguousarray(
            w[rs].astype(np.float32).reshape(rb_n, P).T
        )  # [P, rb_n]
        in_maps.append(
            {
                "A": np.ascontiguousarray(A[rs]),
                "B": np.ascontiguousarray(B[rs]),
                "w": w_core,
                "b8": b8,
            }
        )
    return in_maps


def kernel(A, B, w, b, cw=4096, trace=False):
    A = np.asarray(A, dtype=np.float32)
    B = np.asarray(B, dtype=np.float32)
    w = np.asarray(w, dtype=np.float32)
    b = np.asarray(b, dtype=np.float32)
    d = A.shape[1]
    rows = A.shape[0] // N_CORES
    nc = _get_nc(rows, d, cw, N_CORES)
    in_maps = make_in_maps(A, B, w, b, N_CORES)
    res = run_bass_kernel_spmd(
        nc, in_maps, core_ids=list(range(N_CORES)), trace=trace
    )
    out = np.concatenate([r["out"] for r in res.results], axis=0)
    if trace:
        kernel.last_results = res
    return out


# revision 38
# speedup vs baseline: 1.2677x; 1.2474x over previous
"""Trainium2 Bass kernel for AttentionFusion3.

Computes, for A, B in [D, D] (D=8192), w in [D], b in [1]:
    diag = sum(A * B, axis=-1)            # [D] row-wise dot products
    aw   = tanh(dot(diag, w) + b[0])      # scalar gate
    out  = A * aw + B * (1 - aw)

Sharding: rows split across 8 NeuronCores (1024 rows each). Each core:
  pass 1: stream its A/B shard, fused multiply+row-reduce (DVE
          tensor_tensor_reduce), fold in w, partition-reduce via a
          ones-matmul on PE (which also broadcasts the partial to all
          128 partitions and adds b/8), AllReduce the [128,1] partial
          vector across the 8 cores (sum of per-core partials + 8*(b/8)
          = dot + b, replicated on every partition),
  gate:   tanh on ScalarE, 1-gate,
  pass 2: re-stream A/B, out = gate*A + (1-gate)*B via two ScalarE
          scales + one DVE add, store.
"""

import numpy as np

try:
    import concourse.bass as bass
except ImportError:  # fallback for environments without concourse on sys.path
    import sys

    for _p in ("/opt/trn_rl_repo", "/root/.axon_site/_ro/trn_rl_repo"):
        sys.path.insert(0, _p)
    import concourse.bass as bass  # noqa: F401

import concourse.bacc as bacc
import concourse.mybir as mybir
from concourse import tile
from concourse.bass_utils import run_bass_kernel_spmd

N_CORES = 8
P = 128
D = 8192
ROWS = D // N_CORES  # 1024 rows per core


def build_nc(
    rows=ROWS,
    d=D,
    cw=4096,
    n_cores=N_CORES,
    ccin_engine="sync",
    gate_prefetch=False,
    keep_pairs=0,
    abufs=6,
    bbufs=2,
    skip_b=True,
    raw_store=True,
    ubufs=3,
):
    """Build + compile the per-core Bass program.

    rows: rows per core; d: row length; cw: column chunk width.
    """
    rb_n = rows // P  # row blocks per core
    cc_n = d // cw  # column chunks per row block

    fp32 = mybir.dt.float32
    Alu = mybir.AluOpType
    Act = mybir.ActivationFunctionType

    nc = bacc.Bacc(
        "TRN2", target_bir_lowering=False, debug=False, num_devices=n_cores
    )
    A = nc.dram_tensor("A", [rows, d], fp32, kind="ExternalInput")
    Bm = nc.dram_tensor("B", [rows, d], fp32, kind="ExternalInput")
    W = nc.dram_tensor("w", [P, rb_n], fp32, kind="ExternalInput")
    B8 = nc.dram_tensor("b8", [1, 1], fp32, kind="ExternalInput")
    OUT = nc.dram_tensor("out", [rows, d], fp32, kind="ExternalOutput")

    chunks = [(rb, cc) for rb in range(rb_n) for cc in range(cc_n)]
    keep_chunks = chunks[len(chunks) - keep_pairs :] if keep_pairs else []
    keep_set = set(keep_chunks)
    kept_tiles = {}

    with tile.TileContext(nc) as tc:
        with (
            tc.tile_pool(name="a_pool", bufs=abufs) as a_pool,
            tc.tile_pool(name="b_pool", bufs=bbufs) as b_pool,
            tc.tile_pool(name="u_pool", bufs=ubufs) as u_pool,
            tc.tile_pool(name="keep_pool", bufs=1) as keep_pool,
            tc.tile_pool(name="small", bufs=1) as small,
            tc.tile_pool(name="dram", bufs=1, space="DRAM") as dram,
            tc.tile_pool(name="psum", bufs=1, space="PSUM") as psum_pool,
        ):
            w_t = small.tile([P, rb_n], fp32)
            b8_t = small.tile([1, 1], fp32)
            diagcols = small.tile([P, rb_n * cc_n], fp32)
            comb = small.tile([P, rb_n], fp32)
            sred = small.tile([P, 1], fp32)
            ones_t = small.tile([P, P], fp32)
            gathered = small.tile([P, n_cores], fp32)
            pre_gate = small.tile([P, 1], fp32)
            gate = small.tile([P, 1], fp32)
            omg = small.tile([P, 1], fp32)
            cc_sb = small.tile([P, 1], fp32)
            cc_in = dram.tile([P, 1], fp32)
            cc_out = dram.tile([P * n_cores, 1], fp32, addr_space="Shared")
            ps = psum_pool.tile([P, 1], fp32)

            # tiny setup transfers go on Scalar's DGE so the Sync queue's
            # first entries are the pass-1 bulk loads
            nc.scalar.dma_start(w_t[:], W[:])
            nc.scalar.dma_start(b8_t[:], B8[:])
            nc.vector.memset(ones_t[:], 1.0)

            # ---- pass 1: diagcols[:, cc*rb_n + rb] = rowsum(A_chunk * B_chunk)
            for rb, cc in chunks:
                kept = (rb, cc) in keep_set
                if kept:
                    a_t = keep_pool.tile([P, cw], fp32, tag="ka", bufs=keep_pairs)
                    b_t = keep_pool.tile([P, cw], fp32, tag="kb", bufs=keep_pairs)
                    kept_tiles[(rb, cc)] = (a_t, b_t)
                    # product must not clobber the retained A data; share the
                    # regular a_t slot rotation rather than adding a new tag
                    prod = a_pool.tile([P, cw], fp32, tag="a_t")
                else:
                    a_t = a_pool.tile([P, cw], fp32)
                    b_t = b_pool.tile([P, cw], fp32)
                    prod = a_t
                rs = slice(rb * P, (rb + 1) * P)
                cs = slice(cc * cw, (cc + 1) * cw)
                nc.sync.dma_start(a_t[:], A[rs, cs])
                nc.sync.dma_start(b_t[:], Bm[rs, cs])
                k = cc * rb_n + rb
                # fused A*B + row-sum in one DVE op (product written
                # in-place over the A tile and discarded)
                nc.vector.scalar_tensor_tensor(
                    out=prod[:],
                    in0=a_t[:],
                    scalar=1.0,
                    in1=b_t[:],
                    op0=Alu.mult,
                    op1=Alu.mult,
                    accum_out=diagcols[:, k : k + 1],
                )

            # combine column chunks: comb[:, rb] = sum_cc diagcols[:, cc*rb_n+rb]
            if cc_n == 1:
                nc.vector.tensor_copy(comb[:], diagcols[:])
            else:
                nc.vector.tensor_add(
                    comb[:], diagcols[:, 0:rb_n], diagcols[:, rb_n : 2 * rb_n]
                )
                for cc in range(2, cc_n):
                    nc.vector.tensor_add(
                        comb[:], comb[:], diagcols[:, cc * rb_n : (cc + 1) * rb_n]
                    )
            # fold in w and reduce to per-partition partial
            nc.vector.tensor_mul(comb[:], comb[:], w_t[:])
            nc.vector.reduce_sum(sred[:], comb[:], axis=mybir.AxisListType.X)

            # partition-sum + broadcast via ones-matmul; accumulate b/8
            nc.tensor.matmul(ps[:], ones_t[:], sred[:], start=True, stop=False)
            nc.tensor.matmul(ps[:], ones_t[0:1, :], b8_t[:], start=False, stop=True)
            nc.scalar.copy(cc_sb[:], ps[:])

            # AllGather the [128,1] partials (all lanes identical per core):
            # out is [n_cores*128, 1] with rank r's block at rows r*128..;
            # AG has a much lower latency floor than AllReduce, and the
            # 8-way sum is a trivial local reduce.
            # cc_in is issued off the Sync DGE: Sync's queues are full of
            # bulk loads, which would delay this tiny transfer ~30us.
            ccin_eng = getattr(nc, ccin_engine)
            ccin_inst = ccin_eng.dma_start(cc_in[:], cc_sb[:])
            nc.gpsimd.collective_compute(
                "AllGather",
                Alu.bypass,
                replica_groups=[list(range(n_cores))],
                ins=[cc_in.opt()],
                outs=[cc_out.opt()],
            )
            # gathered[p, r] = cc_out[r*128 + p]. Issued from the Scalar
            # sequencer: it waits on the AllGather, and on Sync it would
            # head-of-line-block every pass-2 load queued behind it; Scalar's
            # next op (tanh) depends on this load anyway.
            nc.scalar.dma_start(
                gathered[:], cc_out[:, 0].rearrange("(r p) -> p r", p=P)
            )
            nc.vector.reduce_sum(
                pre_gate[:], gathered[:], axis=mybir.AxisListType.X
            )

            nc.scalar.activation(gate[:], pre_gate[:], Act.Tanh)
            # omg = 1 - gate  (Copy computes scale*in + bias)
            nc.scalar.activation(omg[:], gate[:], Act.Copy, bias=1.0, scale=-1.0)

            blend_cond = None
            if skip_b:
                # When the gate saturates to exactly 1.0f (the typical case
                # for large random inputs: tanh(|x|>9) rounds to 1.0), the
                # blend is out = 1.0*A + 0.0*B, which is bit-identical to
                # computing it with ANY finite stale B tile. So predicate the
                # pass-2 B loads on gate != 1.0 — when saturated, the DMAs
                # are skipped on-device (semaphores still fire) and pass 2
                # only streams A. The stale b_t slots hold pass-1 B data, so
                # they are always finite and 0.0*stale == 0.0.
                gate_bits = nc.values_load(
                    gate.bitcast(mybir.dt.uint32)[0:1, 0:1],
                    engines=(mybir.EngineType.SP, mybir.EngineType.Activation),
                )
                blend_cond = gate_bits != 0x3F800000  # 1.0f bit pattern
                sat_cond = gate_bits == 0x3F800000

            # ---- pass 2: out = gate*A + (1-gate)*B
            # Retained chunks first: their data is already in SBUF, so the
            # moment the gate lands they compute + store with zero DMA loads.
            first_pass2_load = True
            pass2_order = keep_chunks + [c for c in chunks if c not in keep_set]
            for rb, cc in pass2_order:
                rs = slice(rb * P, (rb + 1) * P)
                cs = slice(cc * cw, (cc + 1) * cw)
                if (rb, cc) in keep_set:
                    a_t, b_t = kept_tiles[(rb, cc)]
                else:
                    a_t = a_pool.tile([P, cw], fp32)
                    b_t = b_pool.tile([P, cw], fp32)
                    la = nc.sync.dma_start(a_t[:], A[rs, cs])
                    if blend_cond is not None:
                        # GpSimd (otherwise idle) zeroes the tile so the
                        # skipped-load case reads defined data: omg*0 == 0.
                        nc.gpsimd.memset(b_t[:], 0.0)
                        lb = nc.sync.dma_start(
                            b_t[:], Bm[rs, cs], cond=blend_cond, cond_hint=False
                        )
                    else:
                        lb = nc.sync.dma_start(b_t[:], Bm[rs, cs])
                    if first_pass2_load and gate_prefetch:
                        # Hold pass-2 prefetch until the tiny collective-input
                        # DMA has landed: otherwise ~20MB of queued prefetch
                        # descriptors delay cc_in (and thus every rank's
                        # arrival at the AllGather) by ~40us. The Sync stream
                        # is in-order, so gating the first pair gates all.
                        import bass_rust as _br

                        _br.add_dep_helper(
                            la.ins, ccin_inst.ins, sync=True,
                            reason="drain DGE backlog before collective input",
                        )
                        _br.add_dep_helper(
                            lb.ins, ccin_inst.ins, sync=True,
                            reason="drain DGE backlog before collective input",
                        )
                        first_pass2_load = False
                if raw_store and blend_cond is not None and (rb, cc) not in keep_set:
                    # Saturated fast path: out == A bit-exactly, so store the
                    # raw A tile gated only on its load — pass 2 becomes a
                    # pure DMA stream and the blend chain (still computed,
                    # into a scratch tile) falls off the critical path.
                    u_t = u_pool.tile([P, cw], fp32)
                    nc.sync.dma_start(OUT[rs, cs], a_t[:], cond=sat_cond)
                    nc.scalar.mul(u_t[:], a_t[:], gate[:])
                    nc.scalar.mul(b_t[:], b_t[:], omg[:])
                    nc.vector.tensor_add(u_t[:], u_t[:], b_t[:])
                    # blend store via Scalar's DGE: when skipped (saturated
                    # case) it must not sit in the Sync queue, where its wait
                    # on the DVE add would head-of-line-block the next raw
                    # store and throttle stores to compute cadence.
                    nc.scalar.dma_start(OUT[rs, cs], u_t[:], cond=blend_cond)
                else:
                    nc.scalar.mul(a_t[:], a_t[:], gate[:])
                    nc.scalar.mul(b_t[:], b_t[:], omg[:])
                    nc.vector.tensor_add(a_t[:], a_t[:], b_t[:])
                    nc.sync.dma_start(OUT[rs, cs], a_t[:])

    nc.compile()
    return nc


_NC_CACHE = {}


def _get_nc(rows, d, cw, n_cores, **cfg):
    key = (rows, d, cw, n_cores, tuple(sorted(cfg.items())))
    if key not in _NC_CACHE:
        _NC_CACHE[key] = build_nc(rows, d, cw, n_cores, **cfg)
    return _NC_CACHE[key]


def make_in_maps(A, B, w, b, n_cores=N_CORES):
    """Shard full inputs row-wise into per-core input maps."""
    rows = A.shape[0] // n_cores
    rb_n = rows // P
    b8 = (b.astype(np.float32) / n_cores).reshape(1, 1)
    in_maps = []
    for c in range(n_cores):
        rs = slice(c * rows, (c + 1) * rows)
        w_core = np.ascontiguousarray(
            w[rs].astype(np.float32).reshape(rb_n, P).T
        )  # [P, rb_n]
        in_maps.append(
            {
                "A": np.ascontiguousarray(A[rs]),
                "B": np.ascontiguousarray(B[rs]),
                "w": w_core,
                "b8": b8,
            }
        )
    return in_maps


def kernel(A, B, w, b, cw=4096, trace=False, **cfg):
    A = np.asarray(A, dtype=np.float32)
    B = np.asarray(B, dtype=np.float32)
    w = np.asarray(w, dtype=np.float32)
    b = np.asarray(b, dtype=np.float32)
    d = A.shape[1]
    rows = A.shape[0] // N_CORES
    nc = _get_nc(rows, d, cw, N_CORES, **cfg)
    in_maps = make_in_maps(A, B, w, b, N_CORES)
    res = run_bass_kernel_spmd(
        nc, in_maps, core_ids=list(range(N_CORES)), trace=trace
    )
    out = np.concatenate([r["out"] for r in res.results], axis=0)
    if trace:
        kernel.last_results = res
    return out
